# revision 7
# baseline (speedup 1.0000x reference)
"""Causal self-attention (RoPE) Trainium2 kernel, 8-core SPMD.

Sharding: core c -> (batch b = c//2, head-group g = c%2). Each core computes
its batch's attention output restricted to its 8 heads, then applies the
corresponding 512 rows of Wo^T; the host sums the two head-group partials
per batch (Megatron-style row-parallel output projection).

Device layout is fully transposed (features on partitions):
  XT  (D=1024, S=2048)                    streamed in s-chunks
  QT/KT = Wg @ X^T  (512, S)              RoPE applied via PE rotation matmul
  scoresT = K @ Q^T (s2 parts, s1 free)   exp on ScalarE, causal blocks only
  OT_aug = [V | 1]^T @ P^T (65, s1)       row 64 = softmax denominator
  partial = OT^T @ Wo_g^T                 accumulated over 4 K-tiles on PE
"""

import math

import numpy as np

B, S, DIM = 4, 2048, 1024
NUM_HEADS = 16
HEAD_DIM = 64
ROPE_BASE = 10000.0
N_CORES = 8
HG = 8          # heads per core (head-group)
O = HG * HEAD_DIM  # 512 per-core projection width
C1 = 1024       # phase-2 s1 chunk width
F32MAX = np.float32

_NC = None  # cached compiled Bass program


def _rope_tables():
    inv_freq = 1.0 / (ROPE_BASE ** (np.arange(0, HEAD_DIM, 2, dtype=np.float64) / HEAD_DIM))
    t = np.arange(S, dtype=np.float64)
    freqs = np.einsum("i,j->ij", t, inv_freq)          # (S, 32)
    emb = np.concatenate([freqs, freqs], axis=-1)      # (S, 64)
    cos = np.cos(emb).astype(np.float32)
    sin = np.sin(emb).astype(np.float32)
    # transposed + tiled to 128 partitions (2 heads per 128-row tile)
    cosT = np.tile(cos.T, (2, 1))                      # (128, S)
    sinT = np.tile(sin.T, (2, 1))
    return cosT, sinT


def _rot_matrix():
    # rotate_half as a matrix: out[d] = -q[d+32] (d<32), q[d-32] (d>=32)
    r = np.zeros((HEAD_DIM, HEAD_DIM), dtype=np.float32)
    for d in range(32):
        r[d, d + 32] = -1.0
        r[d + 32, d] = 1.0
    r128 = np.zeros((128, 128), dtype=np.float32)
    r128[:64, :64] = r
    r128[64:, 64:] = r
    return r128.T.copy()  # lhsT for out = R @ q


def _build_nc():
    from contextlib import ExitStack

    import concourse.mybir as mybir
    import concourse.tile as tile
    from concourse import bacc

    f32 = mybir.dt.float32
    f32r = mybir.dt.float32r

    nc = bacc.Bacc("TRN2", target_bir_lowering=False, debug=False,
                   num_devices=N_CORES)

    xT = nc.declare_dram_parameter("xT", [DIM, S], f32r, isOutput=False)
    wqT = nc.declare_dram_parameter("wqT", [DIM, O], f32r, isOutput=False)
    wkT = nc.declare_dram_parameter("wkT", [DIM, O], f32r, isOutput=False)
    wvT = nc.declare_dram_parameter("wvT", [DIM, O], f32r, isOutput=False)
    woT = nc.declare_dram_parameter("woT", [O, DIM], f32r, isOutput=False)
    cosT = nc.declare_dram_parameter("cosT", [128, S], f32, isOutput=False)
    sinT = nc.declare_dram_parameter("sinT", [128, S], f32, isOutput=False)
    rT = nc.declare_dram_parameter("rT", [128, 128], f32r, isOutput=False)
    dmask = nc.declare_dram_parameter("dmask", [128, 128], f32r, isOutput=False)
    out = nc.declare_dram_parameter("out", [S, DIM], f32, isOutput=True)

    xT3 = xT.ap().rearrange("(o p) s -> p o s", p=128)      # (128, 8, S)
    wq3 = wqT.ap().rearrange("(o p) f -> p o f", p=128)     # (128, 8, 512)
    wk3 = wkT.ap().rearrange("(o p) f -> p o f", p=128)
    wv3 = wvT.ap().rearrange("(o p) f -> p o f", p=128)
    wo3 = woT.ap().rearrange("(o p) f -> p o f", p=128)     # (128, 4, 1024)

    def r(ap):
        return ap

    with tile.TileContext(nc) as tc, ExitStack() as top:
        ot_pool = top.enter_context(tc.tile_pool(name="ot", bufs=1))
        OT = ot_pool.tile([128, 4, S], f32r)   # normalized attn out, transposed

        with ExitStack() as mid:
            qk_pool = mid.enter_context(tc.tile_pool(name="qk", bufs=1))
            QT = qk_pool.tile([128, 4, S], f32r)
            KT = qk_pool.tile([128, 4, S], f32r)
            VA = qk_pool.tile([128, 16, 520], f32r)  # [V(64) | ones] per head

            # ---------------- phase 1: projections + RoPE ----------------
            with ExitStack() as ph1:
                const1 = ph1.enter_context(tc.tile_pool(name="const1", bufs=1))
                xp = ph1.enter_context(tc.tile_pool(name="xp", bufs=2))
                wp = ph1.enter_context(tc.tile_pool(name="wp", bufs=1))
                tp = ph1.enter_context(tc.tile_pool(name="tp", bufs=2))
                rawp = ph1.enter_context(tc.tile_pool(name="rawp", bufs=2))
                ps1 = ph1.enter_context(
                    tc.tile_pool(name="ps1", bufs=2, space="PSUM"))
                psr = ph1.enter_context(
                    tc.tile_pool(name="psr", bufs=2, space="PSUM"))

                cos_sb = const1.tile([128, S], f32)
                sin_sb = const1.tile([128, S], f32)
                rt_sb = const1.tile([128, 128], f32r)
                nc.sync.dma_start(cos_sb[:], cosT.ap())
                nc.sync.dma_start(sin_sb[:], sinT.ap())
                nc.sync.dma_start(rt_sb[:], rT.ap())

                for w3, dest in ((wq3, QT), (wk3, KT), (wv3, None)):
                    w_sb = wp.tile([128, 8, O], f32r, tag="w")
                    nc.sync.dma_start(w_sb[:], w3)
                    for sc in range(4):
                        x_sb = xp.tile([128, 8, 512], f32r, tag="x")
                        nc.sync.dma_start(
                            x_sb[:], xT3[:, :, sc * 512:(sc + 1) * 512])
                        if dest is not None:
                            # QT/KT: (feat parts, s free) + RoPE
                            for ot in range(4):
                                acc = ps1.tile([128, 512], f32, tag="ps1")
                                for kt in range(8):
                                    nc.tensor.matmul(
                                        acc[:],
                                        r(w_sb[:, kt, ot * 128:(ot + 1) * 128]),
                                        r(x_sb[:, kt, :]),
                                        start=(kt == 0), stop=(kt == 7))
                                raw = rawp.tile([128, 512], f32r, tag="raw")
                                nc.scalar.copy(raw[:], acc[:])
                                rot = psr.tile([128, 512], f32, tag="rot")
                                nc.tensor.matmul(rot[:], r(rt_sb[:]), r(raw[:]),
                                                 start=True, stop=True)
                                sl = slice(sc * 512, (sc + 1) * 512)
                                t1 = tp.tile([128, 512], f32, tag="t1")
                                nc.vector.tensor_mul(t1[:], acc[:], cos_sb[:, sl])
                                t2 = tp.tile([128, 512], f32, tag="t2")
                                nc.vector.tensor_mul(t2[:], rot[:], sin_sb[:, sl])
                                nc.vector.tensor_add(dest[:, ot, sl], t1[:], t2[:])
                        else:
                            # V: natural layout (s2 parts, feat free), 65-strided
                            for st in range(4):
                                s2t = sc * 4 + st
                                acc = ps1.tile([128, 512], f32, tag="ps1")
                                for kt in range(8):
                                    nc.tensor.matmul(
                                        acc[:],
                                        r(x_sb[:, kt, st * 128:(st + 1) * 128]),
                                        r(w_sb[:, kt, :]),
                                        start=(kt == 0), stop=(kt == 7))
                                vsl = VA[:, s2t, :].rearrange(
                                    "p (h c) -> p h c", c=65)
                                nc.vector.tensor_copy(
                                    vsl[:, :, 0:64],
                                    acc[:].rearrange("p (h c) -> p h c", c=64))
                                nc.vector.tensor_scalar(
                                    vsl[:, :, 64:65], vsl[:, :, 64:65],
                                    0.0, 1.0, mybir.AluOpType.mult,
                                    mybir.AluOpType.add)

            # ---------------- phase 2: attention ----------------
            with ExitStack() as ph2:
                const2 = ph2.enter_context(tc.tile_pool(name="const2", bufs=1))
                ep = ph2.enter_context(tc.tile_pool(name="ep", bufs=4))
                bp = ph2.enter_context(tc.tile_pool(name="bp", bufs=2))
                rp = ph2.enter_context(tc.tile_pool(name="rp", bufs=2))
                pss = ph2.enter_context(
                    tc.tile_pool(name="pss", bufs=2, space="PSUM"))
                pso = ph2.enter_context(
                    tc.tile_pool(name="pso", bufs=2, space="PSUM"))

                dm_sb = const2.tile([128, 128], f32r)
                nc.sync.dma_start(dm_sb[:], dmask.ap())

                for ot in range(4):           # head pair (QT/KT 128-row tile)
                    for c in range(2):        # s1 chunk of 1024
                        otps = {}
                        for hb in (0, 64):
                            otps[hb] = pso.tile([65, C1], f32, tag="otps", name=f"otps{hb}")
                        for j in range(8 * c + 8):      # s2 blocks of 128
                            l0 = max(0, 128 * j - C1 * c)
                            for hb in (0, 64):
                                h = 2 * ot + (1 if hb else 0)
                                sc_ps = pss.tile([128, C1], f32, tag="sc")
                                for n in range(2):
                                    if 512 * (n + 1) <= l0:
                                        continue
                                    nc.tensor.matmul(
                                        sc_ps[:, n * 512:(n + 1) * 512],
                                        r(KT[hb:hb + 64, ot,
                                             j * 128:(j + 1) * 128]),
                                        r(QT[hb:hb + 64, ot,
                                             c * C1 + n * 512:
                                             c * C1 + (n + 1) * 512]),
                                        start=True, stop=True)
                                et = ep.tile([128, C1], f32r, tag="e")
                                nc.scalar.activation(
                                    et[:, l0:C1], sc_ps[:, l0:C1],
                                    mybir.ActivationFunctionType.Exp,
                                    scale=1.0 / math.sqrt(HEAD_DIM))
                                if 128 * j >= C1 * c:
                                    dl = 128 * j - C1 * c
                                    nc.gpsimd.tensor_mul(
                                        et[:, dl:dl + 128],
                                        et[:, dl:dl + 128], dm_sb[:])
                                for n in range(2):
                                    if 512 * (n + 1) <= l0:
                                        continue
                                    ln = max(l0, 512 * n)
                                    nc.tensor.matmul(
                                        otps[hb][:, ln:512 * (n + 1)],
                                        r(VA[:, j, h * 65:(h + 1) * 65]),
                                        r(et[:, ln:512 * (n + 1)]),
                                        start=(j == 0),
                                        stop=(j == 8 * c + 4 * n + 3))
                        for hb in (0, 64):
                            rec = rp.tile([1, C1], f32, tag="rec")
                            nc.vector.reciprocal(rec[:], otps[hb][64:65, :])
                            bc = bp.tile([64, C1], f32, tag="bc")
                            nc.gpsimd.partition_broadcast(bc[:], rec[:])
                            nc.vector.tensor_mul(
                                OT[hb:hb + 64, ot, c * C1:(c + 1) * C1],
                                otps[hb][0:64, :], bc[:])

        # ---------------- phase 3: output projection ----------------
        with ExitStack() as ph3:
            wop = ph3.enter_context(tc.tile_pool(name="wop", bufs=1))
            stg = ph3.enter_context(tc.tile_pool(name="stg", bufs=3))
            psp = ph3.enter_context(
                tc.tile_pool(name="psp", bufs=4, space="PSUM"))

            wo_sb = wop.tile([128, 4, DIM], f32r)
            nc.sync.dma_start(wo_sb[:], wo3)
            for sb in range(16):
                st = stg.tile([128, DIM], f32, tag="st")
                for half in range(2):
                    acc = psp.tile([128, 512], f32, tag="pp")
                    for kt in range(4):
                        nc.tensor.matmul(
                            acc[:],
                            r(OT[:, kt, sb * 128:(sb + 1) * 128]),
                            r(wo_sb[:, kt, half * 512:(half + 1) * 512]),
                            start=(kt == 0), stop=(kt == 3))
                    nc.vector.tensor_copy(
                        st[:, half * 512:(half + 1) * 512], acc[:])
                nc.sync.dma_start(out.ap()[sb * 128:(sb + 1) * 128, :], st[:])

    nc.compile()
    return nc


def _get_nc():
    global _NC
    if _NC is None:
        _NC = _build_nc()
    return _NC


def make_in_maps(x, Wq, Wk, Wv, Wo):
    cosT, sinT = _rope_tables()
    rT = _rot_matrix()
    # keep where s2 <= s1 in (s2, s1) indexing -> upper-tri incl diag
    dm = np.triu(np.ones((128, 128), dtype=np.float32))
    in_maps = []
    for c in range(N_CORES):
        b, g = c // 2, c % 2
        rows = slice(g * O, (g + 1) * O)
        in_maps.append({
            "xT": np.ascontiguousarray(x[b].T.astype(np.float32)),
            "wqT": np.ascontiguousarray(Wq[rows, :].T.astype(np.float32)),
            "wkT": np.ascontiguousarray(Wk[rows, :].T.astype(np.float32)),
            "wvT": np.ascontiguousarray(Wv[rows, :].T.astype(np.float32)),
            "woT": np.ascontiguousarray(Wo[:, rows].T.astype(np.float32)),
            "cosT": cosT, "sinT": sinT, "rT": rT, "dmask": dm,
        })
    return in_maps


def _numpy_fallback(x, Wq, Wk, Wv, Wo, mask):
    cosT, sinT = _rope_tables()
    cos, sin = cosT[:64].T, sinT[:64].T                      # (S, 64)
    xq = x @ Wq.T
    xk = x @ Wk.T
    xv = x @ Wv.T

    def heads(t):
        return t.reshape(B, S, NUM_HEADS, HEAD_DIM).transpose(0, 2, 1, 3)

    q, k, v = heads(xq), heads(xk), heads(xv)

    def rot(t):
        return np.concatenate([-t[..., 32:], t[..., :32]], axis=-1)

    q = q * cos + rot(q) * sin
    k = k * cos + rot(k) * sin
    sc = np.einsum("bhsd,bhtd->bhst", q, k) / math.sqrt(HEAD_DIM)
    sc = np.where(mask[None, None] == 0, -np.inf, sc)
    sc = sc - sc.max(axis=-1, keepdims=True)
    e = np.exp(sc)
    p = e / e.sum(axis=-1, keepdims=True)
    o = np.einsum("bhst,bhtd->bhsd", p, v)
    o = o.transpose(0, 2, 1, 3).reshape(B, S, DIM)
    return (o @ Wo.T).astype(np.float32)


def kernel(x, Wq, Wk, Wv, Wo, mask):
    x = np.asarray(x)
    mask = np.asarray(mask)
    causal = bool(
        np.array_equal(np.asarray(mask, dtype=np.int64),
                       np.tril(np.ones((S, S), dtype=np.int64))))
    if not causal:
        return _numpy_fallback(
            np.asarray(x, np.float32), np.asarray(Wq, np.float32),
            np.asarray(Wk, np.float32), np.asarray(Wv, np.float32),
            np.asarray(Wo, np.float32), mask)

    from concourse.bass_utils import run_bass_kernel_spmd

    nc = _get_nc()
    in_maps = make_in_maps(x, Wq, Wk, Wv, Wo)
    res = run_bass_kernel_spmd(nc, in_maps, list(range(N_CORES)))
    out = np.empty((B, S, DIM), dtype=np.float32)
    for b in range(B):
        out[b] = res.results[2 * b]["out"] + res.results[2 * b + 1]["out"]
    return out
